# revision 1
# baseline (speedup 1.0000x reference)
"""MiniTransformerLayer on 8 Trainium2 NeuronCores — fp8 DoubleRow edition.

Sharding (as baseline): core c owns tokens [512c, 512(c+1)) and heads
{2c, 2c+1}; 2 AllGathers (LN1 out, fp8) + 2 AllToAlls (attn out, fp8).

Numerics:
  - all attention-side matmuls in fp8e4 DoubleRow (2 K-chunks per
    instruction, 0.5 cyc/row): qkv, scores (64-partition DR with heads
    stacked in partitions 0:64/64:128), attn@V, softmax denominator
    (ones-DR), out_proj.
  - MLP fc1/fc2 as 3-term hi/lo fp8 DR: W*A = Wh*Ah + (Wl*Ah + Wh*Al),
    weights hi/lo prepared on host, activations hi/lo on device.
  - softmax exp split across engines: Act native exp, plus a
    Schraudolph exp2 bit-trick pipeline (DVE f32->int32 convert from
    PSUM, Pool bitcast copy to fp8). The denominator cancels the shared
    approximation bias; measured end-to-end rel err ~4e-3.
  - LayerNorm stats via fp16 ones-matmuls; per-chunk scale/offset via
    rank-1/rank-2 coefficient matmuls (g,b folded).
  - fixed power-of-2 scales: weights x1024 (host), q/k/v fp8 at 16x,
    probs at 1x, attn out at 64x; unscaled in Act copies / stt epilogues.

Schedule notes: every dma_start costs ~630ns of serialized HWDGE
descriptor-generation, so transfers are batched through 3D DRAM
tensors + transposed access patterns (gathered h: 2 DMAs per token
block; attention outputs: 1 DMA per (head, batch); weights: 2 strips
per DMA). qkv for batch-1 token blocks is woven into the (head0,
batch0) attention blocks; attention query-blocks are emitted in
interleaved pairs with den/attn@V matmuls lagging two chunk-pairs
behind the scores so the in-order PE never waits on an in-flight exp;
LN stats matmuls lag their elementwise producers to stay off the
critical path.
"""

import sys

sys.path.insert(0, "/opt/trn_rl_repo")

import numpy as np
import ml_dtypes

import concourse.bass as bass
import concourse.bacc as bacc
import concourse.tile as tile
import concourse.mybir as mybir
from concourse import bass_utils

F8 = mybir.dt.float8e4
F16 = mybir.dt.float16
F32 = mybir.dt.float32
I32 = mybir.dt.int32
AF = mybir.ActivationFunctionType
DR = mybir.MatmulPerfMode.DoubleRow
E4 = ml_dtypes.float8_e4m3

NCORES = 8
B, S, HID, HEADS, D, FFN = 2, 2048, 2048, 16, 128, 4096
TOK = B * S            # 4096 flat tokens
TPC = TOK // NCORES    # 512 tokens per core
HC = HID // 128        # 16 hidden chunks
FFC = FFN // 128       # 32 ffn chunks
NH = HEADS // NCORES   # 2 heads per core
SCALE = 1.0 / float(np.sqrt(D))
EXP_BIAS = -3.0
EPS = 1e-5
WS = 1024.0            # host weight scale
QS = 2.0 ** -6         # psum(1024 q) -> 16 q
MULT, ADD, SUB = (mybir.AluOpType.mult, mybir.AluOpType.add,
                  mybir.AluOpType.subtract)
# Schraudolph exp2 bit trick: int32 bits = trunc(z*A + BC) ~ exp(z) bits
EXP_A = 12102203.161561485          # 2^23 / ln2
EXP_BC = 1064866805.0               # 127*2^23 - 366393
FE_SCALE = EXP_A * SCALE / 256.0
FE_BIAS = EXP_BC + EXP_A * EXP_BIAS

_CACHE = {}


def _emit(nc, single_core=False):
    xT = nc.dram_tensor("xT", [HC, 128, TPC], F16, kind="ExternalInput")
    wq = nc.dram_tensor("wq", [128, HC * 2 * 128], F8, kind="ExternalInput")
    wk = nc.dram_tensor("wk", [128, HC * 2 * 128], F8, kind="ExternalInput")
    wv = nc.dram_tensor("wv", [128, HC * 256], F8, kind="ExternalInput")
    wo = nc.dram_tensor("wo", [HC, 128, HC * 128], F8, kind="ExternalInput")
    wf1 = nc.dram_tensor("wf1", [FFC, 128, HC * 2 * 128], F8,
                         kind="ExternalInput")
    wf2 = nc.dram_tensor("wf2", [HC, 128, FFC * 2 * 128], F8,
                         kind="ExternalInput")
    g1b1 = nc.dram_tensor("g1b1", [2, HID], F16, kind="ExternalInput")
    g2b2 = nc.dram_tensor("g2b2", [2, HID], F16, kind="ExternalInput")
    ropeC = nc.dram_tensor("ropeC", [128, TOK], F16, kind="ExternalInput")
    ropeS = nc.dram_tensor("ropeS", [128, TOK], F16, kind="ExternalInput")
    outT = nc.dram_tensor("outT", [HC, 128, TPC], F32, kind="ExternalOutput")

    rg = [list(range(NCORES))]

    with tile.TileContext(nc) as tc:
        with (
            nc.allow_low_precision(reason="fp8 kernel: quantized by design"),
            tc.tile_pool(name="const", bufs=1) as const,
            tc.tile_pool(name="dram", bufs=1, space="DRAM") as dram,
            tc.tile_pool(name="wop", bufs=2) as wop,
        ):
            ones16 = const.tile([128, 1], F16, tag="on16")
            nc.vector.memset(ones16[:], 1.0)
            ones8q = const.tile([128, 2, 32], F8, tag="on8q")
            nc.vector.memset(ones8q[:], 0.25)
            eps_b = const.tile([1, 1], F32, tag="epsb")
            nc.vector.memset(eps_b[:], EPS)
            zero1_b = const.tile([1, 1], F32, tag="z1b")
            nc.vector.memset(zero1_b[:], 0.0)
            expb_b = const.tile([128, 1], F32, tag="expb")
            nc.vector.memset(expb_b[:], EXP_BIAS)
            g1b1_sb = const.tile([2, HID], F16, tag="g1b1")
            g2b2_sb = const.tile([2, HID], F16, tag="g2b2")
            nc.scalar.dma_start(g1b1_sb[:], g1b1[:])
            nc.scalar.dma_start(g2b2_sb[:], g2b2[:])

            # resident activations
            x16 = const.tile([128, HC, TPC], F16, tag="x16")
            q_sb = const.tile([128, 2, TOK], F8, tag="qsb")
            k_sb = const.tile([128, 2, TOK], F8, tag="ksb")
            v_sb = const.tile([128, TOK // 128, 256], F8, tag="vsb")
            at_all = const.tile([128, HC, TPC], F8, tag="atall")
            x2_sb = const.tile([128, HC, TPC], F16, tag="x2")

            # collective buffers: [slot/jj, partition, token] 3D layouts so
            # whole slots move in one descriptor-friendly DMA
            ag_in_a = dram.tile([8, 128, TPC], F8)
            ag_in_b = dram.tile([8, 128, TPC], F8)
            a2a_in_m = [dram.tile([NCORES, 128, TPC], F8, name=f"a2ai{m}")
                        for m in range(NH)]
            a2a_out_m = [dram.tile([NCORES, 128, TPC], F8, name=f"a2ao{m}")
                         for m in range(NH)]
            if single_core:
                ag_out_a = dram.tile([NCORES, 8, 128, TPC], F8)
                ag_out_b = dram.tile([NCORES, 8, 128, TPC], F8)
            else:
                ag_out_a = nc.dram_tensor(
                    "ag_out_a_sh", [NCORES, 8, 128, TPC], F8,
                    addr_space="Shared").ap()
                ag_out_b = nc.dram_tensor(
                    "ag_out_b_sh", [NCORES, 8, 128, TPC], F8,
                    addr_space="Shared").ap()

            def _ln_coeff(ps_sx, ps_sq, gb, put, lnp, psbc):
                mu = lnp.tile([1, TPC], F32, tag="mu", bufs=1)
                m2 = lnp.tile([1, TPC], F32, tag="m2", bufs=1)
                var = lnp.tile([1, TPC], F32, tag="var", bufs=1)
                lnv = lnp.tile([1, TPC], F32, tag="lnv", bufs=1)
                rstd16 = lnp.tile([1, TPC], F16, tag="rstd", bufs=1)
                mrs_ones = lnp.tile([2, TPC], F16, tag="mrso", bufs=1)
                nc.vector.tensor_scalar_mul(mu[:], ps_sx[:], 1.0 / HID)
                nc.vector.tensor_scalar_mul(m2[:], ps_sq[:], 1.0 / HID)
                nc.vector.tensor_mul(var[:], mu[:], mu[:])
                nc.vector.tensor_sub(var[:], m2[:], var[:])
                nc.scalar.activation(lnv[:], var[:], AF.Ln, bias=eps_b[:])
                nc.scalar.activation(rstd16[:], lnv[:], AF.Exp,
                                     bias=zero1_b[:], scale=-0.5)
                nc.vector.memset(mrs_ones[:], 1.0)
                nc.vector.tensor_mul(mrs_ones[0:1, :], mu[:], rstd16[:])
                nc.vector.tensor_scalar_mul(mrs_ones[0:1, :],
                                            mrs_ones[0:1, :], -1.0)
                for j in range(HC):
                    cs = slice(j * 128, (j + 1) * 128)
                    ps_c1 = psbc.tile([128, TPC], F32, tag="bc")
                    ps_c0 = psbc.tile([128, TPC], F32, tag="bc")
                    nc.tensor.matmul(ps_c1[:], gb[0:1, cs], rstd16[:],
                                     start=True, stop=True)
                    nc.tensor.matmul(ps_c0[:], gb[0:2, cs], mrs_ones[:],
                                     start=True, stop=True)
                    put(j, ps_c1, ps_c0)

            # ---------------- Stage A: load x, LN1, AllGather ----------------
            with (
                tc.tile_pool(name="lnA", bufs=4) as lnA,
                tc.tile_pool(name="psstA", bufs=2, space="PSUM") as psstA,
                tc.tile_pool(name="psbcA", bufs=4, space="PSUM") as psbcA,
            ):
                for g in range(4):
                    nc.sync.dma_start(
                        x16[:, 4 * g:4 * g + 4, :],
                        xT[4 * g:4 * g + 4, :, :].transpose([1, 0, 2]))
                ps_sx = psstA.tile([1, TPC], F32, tag="st")
                ps_sq = psstA.tile([1, TPC], F32, tag="st")
                sq_tiles = {}
                LAG = 3

                def stats1(j):
                    nc.tensor.matmul(ps_sx[:], ones16[:], x16[:, j, :],
                                     start=(j == 0), stop=(j == HC - 1))
                    nc.tensor.matmul(ps_sq[:], ones16[:], sq_tiles.pop(j)[:],
                                     start=(j == 0), stop=(j == HC - 1))

                for j in range(HC):
                    sqt = lnA.tile([128, TPC], F16, tag="sqt")
                    nc.vector.tensor_mul(sqt[:], x16[:, j, :], x16[:, j, :])
                    sq_tiles[j] = sqt
                    if j >= LAG:
                        stats1(j - LAG)
                for j in range(HC - LAG, HC):
                    stats1(j)

                h1_pair = [None]

                def put_h1(j, ps_c1, ps_c0):
                    # GPSIMD can't read PSUM: DVE does the psum mul, Act
                    # copies c0 to SBUF, Pool does the SBUF-only add.
                    t1 = lnA.tile([128, TPC], F16, tag="t1")
                    c0s = lnA.tile([128, TPC], F16, tag="c0s")
                    nc.vector.tensor_mul(t1[:], x16[:, j, :], ps_c1[:])
                    nc.scalar.activation(c0s[:], ps_c0[:], AF.Copy)
                    if j % 2 == 0:
                        h1_pair[0] = lnA.tile([128, 2, TPC], F8, tag="h1",
                                              bufs=2, name="h1p")
                    h1 = h1_pair[0]
                    nc.gpsimd.tensor_add(h1[:, j % 2, :], t1[:], c0s[:])
                    if j % 2 == 1:
                        u = j // 2
                        half, uu = (ag_in_a, u) if u < 4 else (ag_in_b, u - 4)
                        nc.sync.dma_start(
                            half[2 * uu:2 * uu + 2, :, :].transpose(
                                [1, 0, 2]), h1[:])

                _ln_coeff(ps_sx, ps_sq, g1b1_sb, put_h1, lnA, psbcA)

            if single_core:
                # AllGather stand-in: quarter volume per rank slot (baseline
                # convention) as every-4th-partition-row full-width writes —
                # 512B lines, one DMA per buffer, strided range still covers
                # every slot for dependency tracking.
                for r in range(0, NCORES, 4):
                    nc.sync.dma_start(
                        ag_out_a[r:r + 4, :, ::4, :],
                        ag_in_a[:, ::4, :].unsqueeze(0).to_broadcast(
                            (4, 8, 32, TPC)))
                    nc.sync.dma_start(
                        ag_out_b[r:r + 4, :, ::4, :],
                        ag_in_b[:, ::4, :].unsqueeze(0).to_broadcast(
                            (4, 8, 32, TPC)))
            else:
                nc.gpsimd.collective_compute(
                    "AllGather", mybir.AluOpType.bypass, replica_groups=rg,
                    ins=[ag_in_a.opt()], outs=[ag_out_a.opt()])
                nc.gpsimd.collective_compute(
                    "AllGather", mybir.AluOpType.bypass, replica_groups=rg,
                    ins=[ag_in_b.opt()], outs=[ag_out_b.opt()])

            # ---------------- Stages B+C: qkv + attention (woven) -------------
            SB = S // TPC   # 4 query blocks per batch
            with (
                tc.tile_pool(name="cp", bufs=4) as cp,
                tc.tile_pool(name="ptp", bufs=3) as ptp,
                tc.tile_pool(name="zip", bufs=2) as zip_,
            ):
                def attn_block(m, b, qb, fast_pairs, pools, at4):
                    """fast_pairs kg-indices go through the DVE(+Pool)
                    fast-exp pipeline, the rest through Act exp. den/pv
                    matmuls are emitted 2 pairs behind the scores so the
                    in-order PE never waits on an in-flight exp."""
                    pss_p, pso_p, psdn_p = pools
                    mr = slice(64 * m, 64 * (m + 1))
                    qsl = slice(b * S + qb * TPC, b * S + (qb + 1) * TPC)
                    pso = pso_p.tile([128, TPC], F32, tag="o")
                    psden = psdn_p.tile([32, TPC], F32, tag="dn")
                    pts = {}

                    def score_exp(kg, pool_copy):
                        pss = pss_p.tile([128, 2, TPC], F32, tag="s")
                        for h_ in range(2):
                            kc = kg * 2 + h_
                            ksl = slice(b * S + kc * 128,
                                        b * S + (kc + 1) * 128)
                            nc.tensor.matmul(
                                pss[:, h_, :], k_sb[mr, :, ksl],
                                q_sb[mr, :, qsl],
                                start=True, stop=True, perf_mode=DR)
                        pt = ptp.tile([128, 2, TPC], F8, tag="pt", bufs=6)
                        pts[kg] = pt
                        if kg in fast_pairs:
                            zi = zip_.tile([128, 2, TPC], I32, tag="ziw",
                                           bufs=3)
                            nc.vector.tensor_scalar(
                                zi[:].opt(), pss[:].opt(), FE_SCALE, FE_BIAS,
                                MULT, ADD)
                            ceng = nc.gpsimd if pool_copy else nc.vector
                            ceng.tensor_copy(pt[:].opt(),
                                             zi[:].opt().bitcast(F32))
                        else:
                            nc.scalar.activation(
                                pt[:].opt(), pss[:].opt(), AF.Exp,
                                scale=SCALE / 256.0, bias=expb_b[:])

                    def den_pv(kg):
                        pt = pts.pop(kg)
                        nc.tensor.matmul(
                            psden[:], ones8q[:], pt[:],
                            start=(kg == 0), stop=(kg == 7), perf_mode=DR)
                        nc.tensor.matmul(
                            pso[:],
                            v_sb[:, b * 16 + 2 * kg:b * 16 + 2 * kg + 2,
                                 m * 128:(m + 1) * 128],
                            pt[:],
                            start=(kg == 0), stop=(kg == 7), perf_mode=DR)

                    nf = 0
                    for kg in range(8):
                        if kg in fast_pairs:
                            nf += 1
                        score_exp(kg, pool_copy=(nf % 2 == 1))
                        if kg >= 2:
                            den_pv(kg - 2)
                    den_pv(6)
                    den_pv(7)
                    rec16 = cp.tile([1, TPC], F16, tag="rec")
                    nc.vector.reciprocal(rec16[:], psden[0:1, :])
                    rb = cp.tile([128, TPC], F16, tag="rb")
                    nc.gpsimd.partition_broadcast(rb[:], rec16[:])
                    nc.vector.tensor_mul(at4[:, qb, :], pso[:], rb[:])
                    if qb == SB - 1:
                        nc.sync.dma_start(
                            a2a_in_m[m][b * SB:(b + 1) * SB, :, :].transpose(
                                [1, 0, 2]), at4[:])

                def attn_two(m, b, qb0, fast_pairs, pools, at4):
                    """Interleave two query blocks so engines always have
                    independent work between dependency hops."""
                    pss_p, pso_p, psdn_p = pools
                    mr = slice(64 * m, 64 * (m + 1))
                    ctx = []
                    for qb in (qb0, qb0 + 1):
                        qsl = slice(b * S + qb * TPC, b * S + (qb + 1) * TPC)
                        pso = pso_p.tile([128, TPC], F32, tag="o")
                        psden = psdn_p.tile([32, TPC], F32, tag="dn")
                        ctx.append((qb, qsl, pso, psden, {}))

                    cnt = [0]

                    def score_exp(ci, kg):
                        # per-kc pss tiles (1 PSUM bank each) keep the
                        # scores->exp ring turning at full rate
                        qb, qsl, pso, psden, pts = ctx[ci]
                        pt = ptp.tile([128, 2, TPC], F8, tag="pt", bufs=6)
                        pts[kg] = pt
                        for h_ in range(2):
                            kc = kg * 2 + h_
                            ksl = slice(b * S + kc * 128,
                                        b * S + (kc + 1) * 128)
                            pss = pss_p.tile([128, TPC], F32, tag="s")
                            nc.tensor.matmul(
                                pss[:], k_sb[mr, :, ksl], q_sb[mr, :, qsl],
                                start=True, stop=True, perf_mode=DR)
                            cnt[0] += 1
                            if kg in fast_pairs:
                                zi = zip_.tile([128, TPC], I32, tag="zi",
                                               bufs=4)
                                nc.vector.tensor_scalar(
                                    zi[:], pss[:], FE_SCALE, FE_BIAS,
                                    MULT, ADD)
                                ceng = (nc.vector if cnt[0] % 4 == 0
                                        else nc.gpsimd)
                                ceng.tensor_copy(pt[:, h_, :],
                                                 zi[:].bitcast(F32))
                            else:
                                nc.scalar.activation(
                                    pt[:, h_, :], pss[:], AF.Exp,
                                    scale=SCALE / 256.0, bias=expb_b[:])

                    def den_pv(ci, kg):
                        qb, qsl, pso, psden, pts = ctx[ci]
                        pt = pts.pop(kg)
                        nc.tensor.matmul(
                            psden[:], ones8q[:], pt[:],
                            start=(kg == 0), stop=(kg == 7), perf_mode=DR)
                        nc.tensor.matmul(
                            pso[:],
                            v_sb[:, b * 16 + 2 * kg:b * 16 + 2 * kg + 2,
                                 m * 128:(m + 1) * 128],
                            pt[:],
                            start=(kg == 0), stop=(kg == 7), perf_mode=DR)

                    for kg in range(8):
                        for ci in range(2):
                            score_exp(ci, kg)
                        if kg >= 2:
                            den_pv(0, kg - 2)
                            den_pv(1, kg - 2)
                    for kg in (6, 7):
                        den_pv(0, kg)
                        den_pv(1, kg)
                    for ci in range(2):
                        qb, qsl, pso, psden, pts = ctx[ci]
                        rec16 = cp.tile([1, TPC], F16, tag="rec")
                        nc.vector.reciprocal(rec16[:], psden[0:1, :])
                        rb = cp.tile([128, TPC], F16, tag="rb")
                        nc.gpsimd.partition_broadcast(rb[:], rec16[:])
                        nc.vector.tensor_mul(at4[:, qb, :], pso[:], rb[:])
                        if qb == SB - 1:
                            nc.sync.dma_start(
                                a2a_in_m[m][b * SB:(b + 1) * SB, :, :]
                                .transpose([1, 0, 2]), at4[:])

                def a2a(m):
                    if single_core:
                        # two-hop stand-in (baseline convention), halves
                        # pipelined so wire transfers overlap
                        a2a_mid = dram.tile([NCORES, 128, TPC], F8,
                                            name=f"a2am{m}")
                        for hh in range(2):
                            sl = slice(4 * hh, 4 * hh + 4)
                            nc.sync.dma_start(a2a_mid[sl].opt(),
                                              a2a_in_m[m][sl].opt())
                        for hh in range(2):
                            sl = slice(4 * hh, 4 * hh + 4)
                            nc.sync.dma_start(a2a_out_m[m][sl].opt(),
                                              a2a_mid[sl].opt())
                    else:
                        nc.gpsimd.collective_compute(
                            "AllToAll", mybir.AluOpType.bypass,
                            replica_groups=rg,
                            ins=[a2a_in_m[m].opt()], outs=[a2a_out_m[m].opt()])

                with (
                    tc.tile_pool(name="bwt", bufs=1) as bwt,
                    tc.tile_pool(name="htc", bufs=4) as htc,
                    tc.tile_pool(name="qkpre", bufs=2) as qkpre,
                    tc.tile_pool(name="ropet", bufs=4) as ropet,
                    tc.tile_pool(name="pssA", bufs=2, space="PSUM") as pssA,
                    tc.tile_pool(name="psoA", bufs=1, space="PSUM") as psoA,
                    tc.tile_pool(name="psdnA", bufs=1, space="PSUM") as psdnA,
                    tc.tile_pool(name="psqk", bufs=1, space="PSUM") as psqk,
                    tc.tile_pool(name="psv", bufs=1, space="PSUM") as psv,
                ):
                    poolsA = (pssA, psoA, psdnA)
                    rC = bwt.tile([128, TOK], F16, tag="rC")
                    rS = bwt.tile([128, TOK], F16, tag="rS")
                    nc.scalar.dma_start(rC[:], ropeC[:])
                    nc.scalar.dma_start(rS[:], ropeS[:])
                    wq_sb = bwt.tile([128, HC, 2, 128], F8, tag="wq")
                    wk_sb = bwt.tile([128, HC, 2, 128], F8, tag="wk")
                    wv_sb = bwt.tile([128, HC, 256], F8, tag="wv")
                    nc.scalar.dma_start(wq_sb[:].opt(), wq[:])
                    nc.scalar.dma_start(wk_sb[:].opt(), wk[:])
                    nc.scalar.dma_start(wv_sb[:].opt(), wv[:])

                    def qkv_tb(tb):
                        # gathered h for this token block: one DMA per half
                        ta = htc.tile([128, 8, TPC], F8, tag="hta")
                        tb_ = htc.tile([128, 8, TPC], F8, tag="htb")
                        nc.sync.dma_start(
                            ta[:], ag_out_a[tb].transpose([1, 0, 2]))
                        nc.sync.dma_start(
                            tb_[:], ag_out_b[tb].transpose([1, 0, 2]))
                        hts = ([ta[:, 2 * u:2 * u + 2, :] for u in range(4)]
                               + [tb_[:, 2 * u:2 * u + 2, :]
                                  for u in range(4)])
                        tcols = slice(tb * TPC, (tb + 1) * TPC)
                        for (w_sb, dst) in ((wq_sb, q_sb), (wk_sb, k_sb)):
                            pre = qkpre.tile([128, 2, TPC], F16, tag="pre")
                            for ch in range(2):
                                ps = psqk.tile([128, TPC], F32, tag="qk")
                                for jp in range(8):
                                    nc.tensor.matmul(
                                        ps[:],
                                        w_sb[:, 2 * jp:2 * jp + 2, ch, :],
                                        hts[jp], start=(jp == 0),
                                        stop=(jp == 7), perf_mode=DR)
                                nc.scalar.activation(pre[:, ch, :], ps[:],
                                                     AF.Copy, scale=QS)
                            # RoPE: 4 muls on DVE, 2 fp8 combines on Pool
                            t1 = ropet.tile([128, TPC], F16, tag="t1")
                            t2 = ropet.tile([128, TPC], F16, tag="t2")
                            t3 = ropet.tile([128, TPC], F16, tag="t3")
                            t4 = ropet.tile([128, TPC], F16, tag="t4")
                            nc.vector.tensor_mul(t1[:], pre[:, 0, :],
                                                 rC[:, tcols])
                            nc.vector.tensor_mul(t2[:], pre[:, 1, :],
                                                 rS[:, tcols])
                            nc.vector.tensor_mul(t3[:], pre[:, 0, :],
                                                 rS[:, tcols])
                            nc.vector.tensor_mul(t4[:], pre[:, 1, :],
                                                 rC[:, tcols])
                            nc.gpsimd.tensor_sub(dst[:, 0, tcols], t1[:],
                                                 t2[:])
                            nc.gpsimd.tensor_add(dst[:, 1, tcols], t3[:],
                                                 t4[:])
                        for mt in range(4):
                            ps = psv.tile([128, 256], F32, tag="v")
                            for jp in range(8):
                                nc.tensor.matmul(
                                    ps[:],
                                    hts[jp][:, :, mt * 128:(mt + 1) * 128],
                                    wv_sb[:, 2 * jp:2 * jp + 2, :],
                                    start=(jp == 0), stop=(jp == 7),
                                    perf_mode=DR)
                            nc.scalar.activation(v_sb[:, tb * 4 + mt, :],
                                                 ps[:], AF.Copy, scale=QS)

                    for tb in range(4):
                        qkv_tb(tb)
                    # weave: batch-0 attention of head 0 x qkv for batch 1.
                    at4 = cp.tile([128, SB, TPC], F8, tag="at4", bufs=2)
                    for qb in range(SB):
                        attn_block(0, 0, qb, {2, 5}, poolsA, at4)
                        qkv_tb(4 + qb)
                # qkv pools closed: wider psum rings for remaining attention
                with (
                    tc.tile_pool(name="pssB", bufs=4, space="PSUM") as pssB,
                    tc.tile_pool(name="psoB", bufs=2, space="PSUM") as psoB,
                    tc.tile_pool(name="psdnB", bufs=2, space="PSUM") as psdnB,
                ):
                    poolsB = (pssB, psoB, psdnB)
                    at4 = cp.tile([128, SB, TPC], F8, tag="at4", bufs=2)
                    for qb in (0, 2):
                        attn_two(0, 1, qb, {1, 3, 5}, poolsB, at4)
                    a2a(0)
                    # prefetch: even at_all chunks + first wo pairs during m1
                    for hh in range(2):
                        nc.sync.dma_start(
                            at_all[:, 4 * hh:4 * hh + 4, :],
                            a2a_out_m[0][4 * hh:4 * hh + 4].transpose(
                                [1, 0, 2]))
                    wo_tiles = {}
                    for mp in range(2):
                        ws = wop.tile([128, 2, HC, 128], F8, tag="wo")
                        nc.scalar.dma_start(
                            ws[:].opt(),
                            wo[2 * mp:2 * mp + 2].transpose([1, 0, 2]))
                        wo_tiles[mp] = ws
                    for b in range(B):
                        at4 = cp.tile([128, SB, TPC], F8, tag="at4", bufs=2)
                        for qb in (0, 2):
                            attn_two(1, b, qb, {1, 3, 5}, poolsB, at4)
                    a2a(1)

            # ------------- Stages D+E ----------------------------------------
            with tc.tile_pool(name="late", bufs=1) as late:
              h2x = late.tile([128, HC, 2, TPC], F8, tag="h2x")
              ffx = late.tile([128, FFC, 2, TPC], F8, tag="ffx")
              with (
                tc.tile_pool(name="wop2", bufs=3) as wop2,
                tc.tile_pool(name="lnD", bufs=4) as lnD,
                tc.tile_pool(name="pso2", bufs=3, space="PSUM") as pso2_p,
                tc.tile_pool(name="psstD", bufs=2, space="PSUM") as psstD,
                tc.tile_pool(name="psbcD", bufs=3, space="PSUM") as psbcD,
              ):
                for hh in range(2):
                    nc.sync.dma_start(
                        at_all[:, 8 + 4 * hh:12 + 4 * hh, :],
                        a2a_out_m[1][4 * hh:4 * hh + 4].transpose([1, 0, 2]))
                ps_sx2 = psstD.tile([1, TPC], F32, tag="st")
                ps_sq2 = psstD.tile([1, TPC], F32, tag="st")
                sq2_tiles = {}
                LAG2 = 2

                def stats2(mo):
                    nc.tensor.matmul(ps_sx2[:], ones16[:], x2_sb[:, mo, :],
                                     start=(mo == 0), stop=(mo == HC - 1))
                    nc.tensor.matmul(ps_sq2[:], ones16[:],
                                     sq2_tiles.pop(mo)[:],
                                     start=(mo == 0), stop=(mo == HC - 1))

                for mo in range(HC):
                    mp, s = divmod(mo, 2)
                    if s == 0:
                        if mp in wo_tiles:
                            ws = wo_tiles.pop(mp)
                        else:
                            ws = wop2.tile([128, 2, HC, 128], F8, tag="wo2")
                            nc.scalar.dma_start(
                                ws[:].opt(),
                                wo[2 * mp:2 * mp + 2].transpose([1, 0, 2]))
                        cur_wo = ws
                    ps = pso2_p.tile([128, TPC], F32, tag="o2")
                    for jp in range(8):
                        nc.tensor.matmul(
                            ps[:], cur_wo[:, s, 2 * jp:2 * jp + 2, :],
                            at_all[:, 2 * jp:2 * jp + 2, :],
                            start=(jp == 0), stop=(jp == 7), perf_mode=DR)
                    # x2 = psum * 2^-16 + x  (wo 1024x, at 64x)
                    nc.vector.scalar_tensor_tensor(
                        x2_sb[:, mo, :], ps[:], 2.0 ** -16, x16[:, mo, :],
                        MULT, ADD)
                    sq2 = lnD.tile([128, TPC], F16, tag="sq2")
                    nc.vector.tensor_mul(sq2[:], x2_sb[:, mo, :],
                                         x2_sb[:, mo, :])
                    sq2_tiles[mo] = sq2
                    if mo >= LAG2:
                        stats2(mo - LAG2)
                for mo in range(HC - LAG2, HC):
                    stats2(mo)

                def put_h2(j, ps_c1, ps_c0):
                    t1 = lnD.tile([128, TPC], F16, tag="t1")
                    c0s = lnD.tile([128, TPC], F16, tag="c0s")
                    h2f = lnD.tile([128, TPC], F16, tag="h2f")
                    nc.vector.tensor_mul(t1[:], x2_sb[:, j, :], ps_c1[:])
                    nc.scalar.activation(c0s[:], ps_c0[:], AF.Copy)
                    nc.gpsimd.tensor_add(h2f[:], t1[:], c0s[:])
                    if j % 2 == 0:
                        nc.scalar.activation(h2x[:, j, 0, :], h2f[:], AF.Copy)
                    else:
                        nc.vector.tensor_copy(h2x[:, j, 0, :], h2f[:])
                    nc.gpsimd.tensor_sub(h2x[:, j, 1, :], h2f[:],
                                         h2x[:, j, 0, :])

                _ln_coeff(ps_sx2, ps_sq2, g2b2_sb, put_h2, lnD, psbcD)

              # ------------- Stage E: MLP -------------------------------------
              with (
                tc.tile_pool(name="wf1p", bufs=3) as wf1p,
                tc.tile_pool(name="wf2p", bufs=3) as wf2p,
                tc.tile_pool(name="mlt", bufs=4) as mlt,
                tc.tile_pool(name="psf1", bufs=3, space="PSUM") as psf1_p,
                tc.tile_pool(name="psf2", bufs=3, space="PSUM") as psf2_p,
              ):
                wf1_tiles = {}

                def wf1_load(mp):
                    ws = wf1p.tile([128, 2, HC, 2, 128], F8, tag="wf1")
                    nc.scalar.dma_start(
                        ws[:].opt(),
                        wf1[2 * mp:2 * mp + 2].transpose([1, 0, 2]))
                    wf1_tiles[mp] = ws

                wf1_load(0)
                wf1_load(1)
                for mo in range(FFC):
                    mp, s = divmod(mo, 2)
                    if s == 0:
                        cur_wf1 = wf1_tiles.pop(mp)
                        if 2 * (mp + 2) < FFC:
                            wf1_load(mp + 2)
                    ps = psf1_p.tile([128, TPC], F32, tag="f1")
                    for jp in range(8):  # hi*hi
                        nc.tensor.matmul(
                            ps[:], cur_wf1[:, s, 2 * jp:2 * jp + 2, 1, :],
                            h2x[:, 2 * jp:2 * jp + 2, 0, :],
                            start=(jp == 0), stop=False, perf_mode=DR)
                    for j in range(HC):  # cross: Wl*Ah + Wh*Al
                        nc.tensor.matmul(
                            ps[:], cur_wf1[:, s, j, :, :], h2x[:, j, :, :],
                            start=False, stop=(j == HC - 1), perf_mode=DR)
                    ff16 = mlt.tile([128, TPC], F16, tag="ff16")
                    nc.scalar.activation(ff16[:], ps[:], AF.Gelu,
                                         scale=1.0 / WS)
                    eng, oeng = ((nc.gpsimd, nc.vector) if mo % 2 == 0
                                 else (nc.vector, nc.gpsimd))
                    eng.tensor_copy(ffx[:, mo, 0, :], ff16[:])
                    oeng.tensor_sub(ffx[:, mo, 1, :], ff16[:],
                                    ffx[:, mo, 0, :])
                out_pair = [None]
                for mo in range(HC):
                    ws = wf2p.tile([128, FFC, 2, 128], F8, tag="wf2")
                    nc.scalar.dma_start(ws[:].opt(), wf2[mo])
                    ps = psf2_p.tile([128, TPC], F32, tag="f2")
                    for jp in range(FFC // 2):  # hi*hi
                        nc.tensor.matmul(
                            ps[:], ws[:, 2 * jp:2 * jp + 2, 1, :],
                            ffx[:, 2 * jp:2 * jp + 2, 0, :],
                            start=(jp == 0), stop=False, perf_mode=DR)
                    for j in range(FFC):  # cross
                        nc.tensor.matmul(
                            ps[:], ws[:, j, :, :], ffx[:, j, :, :],
                            start=False, stop=(j == FFC - 1), perf_mode=DR)
                    if mo % 2 == 0:
                        out_pair[0] = mlt.tile([128, 2, TPC], F32, tag="ot",
                                               bufs=2, name="otp")
                    ot = out_pair[0]
                    nc.vector.scalar_tensor_tensor(
                        ot[:, mo % 2, :], ps[:], 1.0 / WS, x2_sb[:, mo, :],
                        MULT, ADD)
                    if mo == HC - 1:
                        # last pair as two singles: shorter drain tail
                        nc.sync.dma_start(
                            outT[mo - 1, :, :], ot[:, 0, :])
                        nc.sync.dma_start(
                            outT[mo, :, :], ot[:, 1, :])
                    elif mo % 2 == 1:
                        nc.sync.dma_start(
                            outT[mo - 1:mo + 1, :, :].transpose([1, 0, 2]),
                            ot[:])
    return nc


def _build():
    if "nc" in _CACHE:
        return _CACHE["nc"]
    nc = bacc.Bacc(
        "TRN2", target_bir_lowering=False, debug=False,
        enable_asserts=True, num_devices=NCORES,
    )
    _emit(nc)
    nc.compile()
    _CACHE["nc"] = nc
    return nc


def _q8(v):
    return np.asarray(v, np.float32).astype(E4)


def prepare_inputs(x, pe, w_qkv, w_out, w_fc1, w_fc2, g1, b1, g2, b2):
    x = np.asarray(x, np.float32)
    pe = np.asarray(pe, np.float32)
    w_qkv = np.asarray(w_qkv, np.float32)
    w_out = np.asarray(w_out, np.float32)
    w_fc1 = np.asarray(w_fc1, np.float32)
    w_fc2 = np.asarray(w_fc2, np.float32)

    xf = x.reshape(TOK, HID)
    ropeC = np.tile(pe[:, 0::2].T, (2, B)).astype(np.float16)   # [128, TOK]
    ropeS = np.tile(pe[:, 1::2].T, (2, B)).astype(np.float16)
    g1b1 = np.stack([np.asarray(g1, np.float32),
                     np.asarray(b1, np.float32)]).astype(np.float16)
    g2b2 = np.stack([np.asarray(g2, np.float32),
                     np.asarray(b2, np.float32)]).astype(np.float16)

    # out_proj: strips over reordered contraction chunks
    # jj<8 -> head 2*jj (from a2a[0]); jj>=8 -> head 2*(jj-8)+1 (a2a[1])
    head_of = [2 * jj if jj < 8 else 2 * (jj - 8) + 1 for jj in range(HC)]
    wo_h = np.empty((HC, 128, HC * 128), dtype=E4)
    wt = (w_out * WS).astype(np.float32)   # [out, feat]
    for mo in range(HC):
        blk = np.empty((128, HC, 128), np.float32)
        for jj in range(HC):
            h = head_of[jj]
            blk[:, jj, :] = wt[mo * 128:(mo + 1) * 128,
                               h * 128:(h + 1) * 128].T
        wo_h[mo] = _q8(blk.reshape(128, HC * 128))

    def hilo_strips(w, n_strips, kc):
        w = (w * WS).astype(np.float32)
        hi = _q8(w).astype(np.float32)
        lo = _q8(w - hi).astype(np.float32)
        out = np.empty((n_strips, 128, kc * 2 * 128), dtype=E4)
        for mo in range(n_strips):
            rows = slice(mo * 128, (mo + 1) * 128)
            blk = np.empty((128, kc, 2, 128), np.float32)
            wl = lo[rows]; wh = hi[rows]    # [128(col), kc*128]
            blk[:, :, 0, :] = wl.reshape(128, kc, 128).transpose(2, 1, 0)
            blk[:, :, 1, :] = wh.reshape(128, kc, 128).transpose(2, 1, 0)
            out[mo] = _q8(blk.reshape(128, kc * 2 * 128))
        return out

    wf1_h = hilo_strips(w_fc1, FFC, HC)     # [FFC, 128, HC*2*128]
    wf2_h = hilo_strips(w_fc2, HC, FFC)     # [HC, 128, FFC*2*128]

    in_maps = []
    for c in range(NCORES):
        hsl = slice(2 * c * D, (2 * c + 2) * D)

        def qk_lay(rows):
            # rows [256, HID] (2 heads) -> [128(p), HC(j), 2(ch), 128(m,pp)]
            r = (rows * WS).astype(np.float32)
            t = r.reshape(2, 64, 2, HC, 128)       # [m, pp, ch, j, p]
            t = t.transpose(4, 3, 2, 0, 1)          # [p, j, ch, m, pp]
            return _q8(t.reshape(128, HC * 2 * 128))

        qrows = w_qkv[hsl]
        krows = w_qkv[HID + 2 * c * D: HID + (2 * c + 2) * D]
        vrows = w_qkv[2 * HID + 2 * c * D: 2 * HID + (2 * c + 2) * D]
        vv = (vrows * WS).astype(np.float32).reshape(2, 128, HC, 128)
        wv_c = _q8(vv.transpose(3, 2, 0, 1).reshape(128, HC * 256))

        xTc = np.ascontiguousarray(
            xf[c * TPC:(c + 1) * TPC].T).astype(np.float16)
        in_maps.append({
            "xT": xTc.reshape(HC, 128, TPC),
            "wq": qk_lay(qrows), "wk": qk_lay(krows), "wv": wv_c,
            "wo": wo_h, "wf1": wf1_h, "wf2": wf2_h,
            "g1b1": g1b1, "g2b2": g2b2,
            "ropeC": ropeC, "ropeS": ropeS,
        })
    return in_maps


def run(in_maps, **kwargs):
    nc = _build()
    return bass_utils.run_bass_kernel_spmd(
        nc, in_maps, core_ids=list(range(NCORES)), **kwargs
    )


def kernel(x, pe, w_qkv, w_out, w_fc1, w_fc2, g1, b1, g2, b2):
    in_maps = prepare_inputs(x, pe, w_qkv, w_out, w_fc1, w_fc2, g1, b1, g2, b2)
    res = run(in_maps)
    fullT = np.concatenate(
        [res.results[c]["outT"].reshape(HID, TPC) for c in range(NCORES)],
        axis=1)
    return np.ascontiguousarray(fullT.T).reshape(B, S, HID).astype(np.float32)



# revision 10
# speedup vs baseline: 1.0564x; 1.0564x over previous
"""MiniTransformerLayer on 8 Trainium2 NeuronCores — fp8 DoubleRow edition.

Sharding (as baseline): core c owns tokens [512c, 512(c+1)) and heads
{2c, 2c+1}; 2 AllGathers (LN1 out, fp8) + 2 AllToAlls (attn out, fp8).

Numerics:
  - all attention-side matmuls in fp8e4 DoubleRow (2 K-chunks per
    instruction, 0.5 cyc/row): qkv, scores (64-partition DR with heads
    stacked in partitions 0:64/64:128), attn@V, softmax denominator
    (ones-DR), out_proj.
  - MLP fc1/fc2 as 3-term hi/lo fp8 DR: W*A = Wh*Ah + (Wl*Ah + Wh*Al),
    weights hi/lo prepared on host, activations hi/lo on device.
  - softmax exp split across engines: Act native exp, plus a
    Schraudolph exp2 bit-trick pipeline (DVE f32->int32 convert from
    PSUM, Pool bitcast copy to fp8). The denominator cancels the shared
    approximation bias; measured end-to-end rel err ~4e-3.
  - LayerNorm stats via fp16 ones-matmuls; per-chunk scale/offset via
    rank-1/rank-2 coefficient matmuls (g,b folded).
  - fixed power-of-2 scales: weights x1024 (host), q/k/v fp8 at 16x,
    probs at 1x, attn out at 64x; unscaled in Act copies / stt epilogues.

Schedule notes: every dma_start costs ~630ns of serialized HWDGE
descriptor-generation, so transfers are batched through 3D DRAM
tensors + transposed access patterns (gathered h: 2 DMAs per token
block; attention outputs: 1 DMA per (head, batch); weights: 2 strips
per DMA). qkv for batch-1 token blocks is woven into the (head0,
batch0) attention blocks; attention query-blocks are emitted in
interleaved pairs with den/attn@V matmuls lagging two chunk-pairs
behind the scores so the in-order PE never waits on an in-flight exp;
LN stats matmuls lag their elementwise producers to stay off the
critical path.
"""

import sys

sys.path.insert(0, "/opt/trn_rl_repo")

import numpy as np
import ml_dtypes

import concourse.bass as bass
import concourse.bacc as bacc
import concourse.tile as tile
import concourse.mybir as mybir
from concourse import bass_utils

F8 = mybir.dt.float8e4
F16 = mybir.dt.float16
F32 = mybir.dt.float32
I32 = mybir.dt.int32
AF = mybir.ActivationFunctionType
DR = mybir.MatmulPerfMode.DoubleRow
E4 = ml_dtypes.float8_e4m3

NCORES = 8
B, S, HID, HEADS, D, FFN = 2, 2048, 2048, 16, 128, 4096
TOK = B * S            # 4096 flat tokens
TPC = TOK // NCORES    # 512 tokens per core
HC = HID // 128        # 16 hidden chunks
FFC = FFN // 128       # 32 ffn chunks
NH = HEADS // NCORES   # 2 heads per core
SCALE = 1.0 / float(np.sqrt(D))
EXP_BIAS = -3.0
EPS = 1e-5
WS = 1024.0            # host weight scale
QS = 2.0 ** -6         # psum(1024 q) -> 16 q
MULT, ADD, SUB = (mybir.AluOpType.mult, mybir.AluOpType.add,
                  mybir.AluOpType.subtract)
# Schraudolph exp2 bit trick: int32 bits = trunc(z*A + BC) ~ exp(z) bits
EXP_A = 12102203.161561485          # 2^23 / ln2
EXP_BC = 1064866805.0               # 127*2^23 - 366393
FE_SCALE = EXP_A * SCALE / 256.0
FE_BIAS = EXP_BC + EXP_A * EXP_BIAS

_CACHE = {}


def _emit(nc, single_core=False):
    xT = nc.dram_tensor("xT", [HC, 128, TPC], F16, kind="ExternalInput")
    wq = nc.dram_tensor("wq", [128, HC * 2 * 128], F8, kind="ExternalInput")
    wk = nc.dram_tensor("wk", [128, HC * 2 * 128], F8, kind="ExternalInput")
    wv = nc.dram_tensor("wv", [128, HC * 256], F8, kind="ExternalInput")
    wo = nc.dram_tensor("wo", [HC, 128, HC * 128], F8, kind="ExternalInput")
    wf1 = nc.dram_tensor("wf1", [FFC, 128, HC * 2 * 128], F8,
                         kind="ExternalInput")
    wf2 = nc.dram_tensor("wf2", [HC, 128, FFC * 2 * 128], F8,
                         kind="ExternalInput")
    g1b1 = nc.dram_tensor("g1b1", [2, HID], F16, kind="ExternalInput")
    g2b2 = nc.dram_tensor("g2b2", [2, HID], F16, kind="ExternalInput")
    ropeC = nc.dram_tensor("ropeC", [128, TOK], F16, kind="ExternalInput")
    ropeS = nc.dram_tensor("ropeS", [128, TOK], F16, kind="ExternalInput")
    outT = nc.dram_tensor("outT", [HC, 128, TPC], F32, kind="ExternalOutput")

    rg = [list(range(NCORES))]

    with tile.TileContext(nc) as tc:
        with (
            nc.allow_low_precision(reason="fp8 kernel: quantized by design"),
            tc.tile_pool(name="const", bufs=1) as const,
            tc.tile_pool(name="dram", bufs=1, space="DRAM") as dram,
            tc.tile_pool(name="wop", bufs=2) as wop,
        ):
            ones16 = const.tile([128, 1], F16, tag="on16")
            nc.vector.memset(ones16[:], 1.0)
            ones8q = const.tile([128, 2, 32], F8, tag="on8q")
            nc.vector.memset(ones8q[:], 0.25)
            eps_b = const.tile([1, 1], F32, tag="epsb")
            nc.vector.memset(eps_b[:], EPS)
            zero1_b = const.tile([1, 1], F32, tag="z1b")
            nc.vector.memset(zero1_b[:], 0.0)
            expb_b = const.tile([128, 1], F32, tag="expb")
            nc.vector.memset(expb_b[:], EXP_BIAS)
            g1b1_sb = const.tile([2, HID], F16, tag="g1b1")
            g2b2_sb = const.tile([2, HID], F16, tag="g2b2")
            nc.scalar.dma_start(g1b1_sb[:], g1b1[:])
            nc.scalar.dma_start(g2b2_sb[:], g2b2[:])

            # resident activations
            x16 = const.tile([128, HC, TPC], F16, tag="x16")
            q_sb = const.tile([128, 2, TOK], F8, tag="qsb")
            k_sb = const.tile([128, 2, TOK], F8, tag="ksb")
            v_sb = const.tile([128, TOK // 128, 256], F8, tag="vsb")
            at_all = const.tile([128, HC, TPC], F8, tag="atall")
            x2_sb = const.tile([128, HC, TPC], F16, tag="x2")

            # collective buffers: [slot/jj, partition, token] 3D layouts so
            # whole slots move in one descriptor-friendly DMA
            ag_in_a = dram.tile([8, 128, TPC], F8)
            ag_in_b = dram.tile([8, 128, TPC], F8)
            a2a_in_m = [dram.tile([NCORES, 128, TPC], F8, name=f"a2ai{m}")
                        for m in range(NH)]
            a2a_out_m = [dram.tile([NCORES, 128, TPC], F8, name=f"a2ao{m}")
                         for m in range(NH)]
            if single_core:
                ag_out_a = dram.tile([NCORES, 8, 128, TPC], F8)
                ag_out_b = dram.tile([NCORES, 8, 128, TPC], F8)
            else:
                ag_out_a = nc.dram_tensor(
                    "ag_out_a_sh", [NCORES, 8, 128, TPC], F8,
                    addr_space="Shared").ap()
                ag_out_b = nc.dram_tensor(
                    "ag_out_b_sh", [NCORES, 8, 128, TPC], F8,
                    addr_space="Shared").ap()

            def _ln_coeff(ps_sx, ps_sq, gb, put, lnp, psbc):
                mu = lnp.tile([1, TPC], F32, tag="mu", bufs=1)
                m2 = lnp.tile([1, TPC], F32, tag="m2", bufs=1)
                var = lnp.tile([1, TPC], F32, tag="var", bufs=1)
                lnv = lnp.tile([1, TPC], F32, tag="lnv", bufs=1)
                rstd16 = lnp.tile([1, TPC], F16, tag="rstd", bufs=1)
                mrs_ones = lnp.tile([2, TPC], F16, tag="mrso", bufs=1)
                nc.vector.tensor_scalar_mul(mu[:], ps_sx[:], 1.0 / HID)
                nc.vector.tensor_scalar_mul(m2[:], ps_sq[:], 1.0 / HID)
                nc.vector.tensor_mul(var[:], mu[:], mu[:])
                nc.vector.tensor_sub(var[:], m2[:], var[:])
                nc.scalar.activation(lnv[:], var[:], AF.Ln, bias=eps_b[:])
                nc.scalar.activation(rstd16[:], lnv[:], AF.Exp,
                                     bias=zero1_b[:], scale=-0.5)
                nc.vector.memset(mrs_ones[:], 1.0)
                nc.vector.tensor_mul(mrs_ones[0:1, :], mu[:], rstd16[:])
                nc.vector.tensor_scalar_mul(mrs_ones[0:1, :],
                                            mrs_ones[0:1, :], -1.0)
                for j in range(HC):
                    cs = slice(j * 128, (j + 1) * 128)
                    ps_c1 = psbc.tile([128, TPC], F32, tag="bc")
                    ps_c0 = psbc.tile([128, TPC], F32, tag="bc")
                    nc.tensor.matmul(ps_c1[:], gb[0:1, cs], rstd16[:],
                                     start=True, stop=True)
                    nc.tensor.matmul(ps_c0[:], gb[0:2, cs], mrs_ones[:],
                                     start=True, stop=True)
                    put(j, ps_c1, ps_c0)

            # ---------------- Stage A: load x, LN1, AllGather ----------------
            with (
                tc.tile_pool(name="lnA", bufs=4) as lnA,
                tc.tile_pool(name="psstA", bufs=2, space="PSUM") as psstA,
                tc.tile_pool(name="psbcA", bufs=4, space="PSUM") as psbcA,
            ):
                for g in range(4):
                    nc.sync.dma_start(
                        x16[:, 4 * g:4 * g + 4, :],
                        xT[4 * g:4 * g + 4, :, :].transpose([1, 0, 2]))
                ps_sx = psstA.tile([1, TPC], F32, tag="st")
                ps_sq = psstA.tile([1, TPC], F32, tag="st")
                sq_tiles = {}
                LAG = 3

                def stats1(j):
                    nc.tensor.matmul(ps_sx[:], ones16[:], x16[:, j, :],
                                     start=(j == 0), stop=(j == HC - 1))
                    nc.tensor.matmul(ps_sq[:], ones16[:], sq_tiles.pop(j)[:],
                                     start=(j == 0), stop=(j == HC - 1))

                for j in range(HC):
                    sqt = lnA.tile([128, TPC], F16, tag="sqt")
                    nc.vector.tensor_mul(sqt[:], x16[:, j, :], x16[:, j, :])
                    sq_tiles[j] = sqt
                    if j >= LAG:
                        stats1(j - LAG)
                for j in range(HC - LAG, HC):
                    stats1(j)

                h1_pair = [None]

                def put_h1(j, ps_c1, ps_c0):
                    # GPSIMD can't read PSUM: DVE does the psum mul, Act
                    # copies c0 to SBUF, Pool does the SBUF-only add.
                    t1 = lnA.tile([128, TPC], F16, tag="t1")
                    c0s = lnA.tile([128, TPC], F16, tag="c0s")
                    nc.vector.tensor_mul(t1[:], x16[:, j, :], ps_c1[:])
                    nc.scalar.activation(c0s[:], ps_c0[:], AF.Copy)
                    if j % 2 == 0:
                        h1_pair[0] = lnA.tile([128, 2, TPC], F8, tag="h1",
                                              bufs=2, name="h1p")
                    h1 = h1_pair[0]
                    nc.gpsimd.tensor_add(h1[:, j % 2, :], t1[:], c0s[:])
                    if j % 2 == 1:
                        u = j // 2
                        half, uu = (ag_in_a, u) if u < 4 else (ag_in_b, u - 4)
                        nc.sync.dma_start(
                            half[2 * uu:2 * uu + 2, :, :].transpose(
                                [1, 0, 2]), h1[:])

                _ln_coeff(ps_sx, ps_sq, g1b1_sb, put_h1, lnA, psbcA)

            if single_core:
                # AllGather stand-in: quarter volume per rank slot (baseline
                # convention) as every-4th-partition-row full-width writes —
                # 512B lines, one DMA per buffer, strided range still covers
                # every slot for dependency tracking.
                for r in range(0, NCORES, 4):
                    nc.sync.dma_start(
                        ag_out_a[r:r + 4, :, ::4, :],
                        ag_in_a[:, ::4, :].unsqueeze(0).to_broadcast(
                            (4, 8, 32, TPC)))
                    nc.sync.dma_start(
                        ag_out_b[r:r + 4, :, ::4, :],
                        ag_in_b[:, ::4, :].unsqueeze(0).to_broadcast(
                            (4, 8, 32, TPC)))
            else:
                nc.gpsimd.collective_compute(
                    "AllGather", mybir.AluOpType.bypass, replica_groups=rg,
                    ins=[ag_in_a.opt()], outs=[ag_out_a.opt()])
                nc.gpsimd.collective_compute(
                    "AllGather", mybir.AluOpType.bypass, replica_groups=rg,
                    ins=[ag_in_b.opt()], outs=[ag_out_b.opt()])

            # ---------------- Stages B+C: qkv + attention (woven) -------------
            SB = S // TPC   # 4 query blocks per batch
            with (
                tc.tile_pool(name="cp", bufs=4) as cp,
                tc.tile_pool(name="ptp", bufs=3) as ptp,
                tc.tile_pool(name="zip", bufs=2) as zip_,
            ):
                def attn_block(m, b, qb, fast_pairs, pools, at4):
                    """fast_pairs kg-indices go through the DVE(+Pool)
                    fast-exp pipeline, the rest through Act exp. den/pv
                    matmuls are emitted 2 pairs behind the scores so the
                    in-order PE never waits on an in-flight exp."""
                    pss_p, pso_p, psdn_p = pools
                    mr = slice(64 * m, 64 * (m + 1))
                    qsl = slice(b * S + qb * TPC, b * S + (qb + 1) * TPC)
                    pso = pso_p.tile([128, TPC], F32, tag="o")
                    psden = psdn_p.tile([32, TPC], F32, tag="dn")
                    pts = {}

                    def score_exp(kg, pool_copy):
                        pss = pss_p.tile([128, 2, TPC], F32, tag="s")
                        for h_ in range(2):
                            kc = kg * 2 + h_
                            ksl = slice(b * S + kc * 128,
                                        b * S + (kc + 1) * 128)
                            nc.tensor.matmul(
                                pss[:, h_, :], k_sb[mr, :, ksl],
                                q_sb[mr, :, qsl],
                                start=True, stop=True, perf_mode=DR)
                        pt = ptp.tile([128, 2, TPC], F8, tag="pt", bufs=6)
                        pts[kg] = pt
                        if kg in fast_pairs:
                            zi = zip_.tile([128, 2, TPC], I32, tag="ziw",
                                           bufs=3)
                            nc.vector.tensor_scalar(
                                zi[:].opt(), pss[:].opt(), FE_SCALE, FE_BIAS,
                                MULT, ADD)
                            ceng = nc.gpsimd if pool_copy else nc.vector
                            ceng.tensor_copy(pt[:].opt(),
                                             zi[:].opt().bitcast(F32))
                        else:
                            nc.scalar.activation(
                                pt[:].opt(), pss[:].opt(), AF.Exp,
                                scale=SCALE / 256.0, bias=expb_b[:])

                    def den_pv(kg):
                        pt = pts.pop(kg)
                        nc.tensor.matmul(
                            psden[:], ones8q[:], pt[:],
                            start=(kg == 0), stop=(kg == 7), perf_mode=DR)
                        nc.tensor.matmul(
                            pso[:],
                            v_sb[:, b * 16 + 2 * kg:b * 16 + 2 * kg + 2,
                                 m * 128:(m + 1) * 128],
                            pt[:],
                            start=(kg == 0), stop=(kg == 7), perf_mode=DR)

                    nf = 0
                    for kg in range(8):
                        if kg in fast_pairs:
                            nf += 1
                        score_exp(kg, pool_copy=(nf % 2 == 1))
                        if kg >= 2:
                            den_pv(kg - 2)
                    den_pv(6)
                    den_pv(7)
                    rec16 = cp.tile([1, TPC], F16, tag="rec")
                    nc.vector.reciprocal(rec16[:], psden[0:1, :])
                    rb = cp.tile([128, TPC], F16, tag="rb")
                    nc.gpsimd.partition_broadcast(rb[:], rec16[:])
                    nc.vector.tensor_mul(at4[:, qb, :], pso[:], rb[:])
                    if qb == SB - 1:
                        nc.sync.dma_start(
                            a2a_in_m[m][b * SB:(b + 1) * SB, :, :].transpose(
                                [1, 0, 2]), at4[:])

                def attn_two(m, b, qb0, fast_pairs, pools, at4):
                    """Interleave two query blocks so engines always have
                    independent work between dependency hops."""
                    pss_p, pso_p, psdn_p = pools
                    mr = slice(64 * m, 64 * (m + 1))
                    ctx = []
                    for qb in (qb0, qb0 + 1):
                        qsl = slice(b * S + qb * TPC, b * S + (qb + 1) * TPC)
                        pso = pso_p.tile([128, TPC], F32, tag="o")
                        psden = psdn_p.tile([32, TPC], F32, tag="dn")
                        ctx.append((qb, qsl, pso, psden, {}))

                    cnt = [0]

                    def score_exp(ci, kg):
                        # per-kc pss tiles (1 PSUM bank each) keep the
                        # scores->exp ring turning at full rate
                        qb, qsl, pso, psden, pts = ctx[ci]
                        pt = ptp.tile([128, 2, TPC], F8, tag="pt", bufs=6)
                        pts[kg] = pt
                        for h_ in range(2):
                            kc = kg * 2 + h_
                            ksl = slice(b * S + kc * 128,
                                        b * S + (kc + 1) * 128)
                            pss = pss_p.tile([128, TPC], F32, tag="s")
                            nc.tensor.matmul(
                                pss[:], k_sb[mr, :, ksl], q_sb[mr, :, qsl],
                                start=True, stop=True, perf_mode=DR)
                            cnt[0] += 1
                            if kg in fast_pairs:
                                zi = zip_.tile([128, TPC], I32, tag="zi",
                                               bufs=4)
                                nc.vector.tensor_scalar(
                                    zi[:], pss[:], FE_SCALE, FE_BIAS,
                                    MULT, ADD)
                                ceng = (nc.vector if cnt[0] % 4 == 0
                                        else nc.gpsimd)
                                ceng.tensor_copy(pt[:, h_, :],
                                                 zi[:].bitcast(F32))
                            else:
                                nc.scalar.activation(
                                    pt[:, h_, :], pss[:], AF.Exp,
                                    scale=SCALE / 256.0, bias=expb_b[:])

                    def den_pv(ci, kg):
                        qb, qsl, pso, psden, pts = ctx[ci]
                        pt = pts.pop(kg)
                        nc.tensor.matmul(
                            psden[:], ones8q[:], pt[:],
                            start=(kg == 0), stop=(kg == 7), perf_mode=DR)
                        nc.tensor.matmul(
                            pso[:],
                            v_sb[:, b * 16 + 2 * kg:b * 16 + 2 * kg + 2,
                                 m * 128:(m + 1) * 128],
                            pt[:],
                            start=(kg == 0), stop=(kg == 7), perf_mode=DR)

                    for kg in range(8):
                        for ci in range(2):
                            score_exp(ci, kg)
                        if kg >= 2:
                            den_pv(0, kg - 2)
                            den_pv(1, kg - 2)
                    for kg in (6, 7):
                        den_pv(0, kg)
                        den_pv(1, kg)
                    for ci in range(2):
                        qb, qsl, pso, psden, pts = ctx[ci]
                        rec16 = cp.tile([1, TPC], F16, tag="rec")
                        nc.vector.reciprocal(rec16[:], psden[0:1, :])
                        rb = cp.tile([128, TPC], F16, tag="rb")
                        nc.gpsimd.partition_broadcast(rb[:], rec16[:])
                        nc.vector.tensor_mul(at4[:, qb, :], pso[:], rb[:])
                        if qb == SB - 1:
                            nc.sync.dma_start(
                                a2a_in_m[m][b * SB:(b + 1) * SB, :, :]
                                .transpose([1, 0, 2]), at4[:])

                def a2a(m):
                    if single_core:
                        # two-hop stand-in (baseline convention), halves
                        # pipelined so wire transfers overlap
                        a2a_mid = dram.tile([NCORES, 128, TPC], F8,
                                            name=f"a2am{m}")
                        for hh in range(2):
                            sl = slice(4 * hh, 4 * hh + 4)
                            nc.sync.dma_start(a2a_mid[sl].opt(),
                                              a2a_in_m[m][sl].opt())
                        for hh in range(2):
                            sl = slice(4 * hh, 4 * hh + 4)
                            nc.sync.dma_start(a2a_out_m[m][sl].opt(),
                                              a2a_mid[sl].opt())
                    else:
                        nc.gpsimd.collective_compute(
                            "AllToAll", mybir.AluOpType.bypass,
                            replica_groups=rg,
                            ins=[a2a_in_m[m].opt()], outs=[a2a_out_m[m].opt()])

                with (
                    tc.tile_pool(name="bwt", bufs=1) as bwt,
                    tc.tile_pool(name="htc", bufs=4) as htc,
                    tc.tile_pool(name="qkpre", bufs=2) as qkpre,
                    tc.tile_pool(name="ropet", bufs=4) as ropet,
                    tc.tile_pool(name="pssA", bufs=2, space="PSUM") as pssA,
                    tc.tile_pool(name="psoA", bufs=1, space="PSUM") as psoA,
                    tc.tile_pool(name="psdnA", bufs=1, space="PSUM") as psdnA,
                    tc.tile_pool(name="psqk", bufs=1, space="PSUM") as psqk,
                    tc.tile_pool(name="psv", bufs=1, space="PSUM") as psv,
                ):
                    poolsA = (pssA, psoA, psdnA)
                    rC = bwt.tile([128, TOK], F16, tag="rC")
                    rS = bwt.tile([128, TOK], F16, tag="rS")
                    nc.scalar.dma_start(rC[:], ropeC[:])
                    nc.scalar.dma_start(rS[:], ropeS[:])
                    wq_sb = bwt.tile([128, HC, 2, 128], F8, tag="wq")
                    wk_sb = bwt.tile([128, HC, 2, 128], F8, tag="wk")
                    wv_sb = bwt.tile([128, HC, 256], F8, tag="wv")
                    nc.scalar.dma_start(wq_sb[:].opt(), wq[:])
                    nc.scalar.dma_start(wk_sb[:].opt(), wk[:])
                    nc.scalar.dma_start(wv_sb[:].opt(), wv[:])

                    def qkv_tb(tb):
                        # gathered h for this token block: one DMA per half
                        ta = htc.tile([128, 8, TPC], F8, tag="hta")
                        tb_ = htc.tile([128, 8, TPC], F8, tag="htb")
                        nc.sync.dma_start(
                            ta[:], ag_out_a[tb].transpose([1, 0, 2]))
                        nc.sync.dma_start(
                            tb_[:], ag_out_b[tb].transpose([1, 0, 2]))
                        hts = ([ta[:, 2 * u:2 * u + 2, :] for u in range(4)]
                               + [tb_[:, 2 * u:2 * u + 2, :]
                                  for u in range(4)])
                        tcols = slice(tb * TPC, (tb + 1) * TPC)
                        for (w_sb, dst) in ((wq_sb, q_sb), (wk_sb, k_sb)):
                            pre = qkpre.tile([128, 2, TPC], F16, tag="pre")
                            for ch in range(2):
                                ps = psqk.tile([128, TPC], F32, tag="qk")
                                for jp in range(8):
                                    nc.tensor.matmul(
                                        ps[:],
                                        w_sb[:, 2 * jp:2 * jp + 2, ch, :],
                                        hts[jp], start=(jp == 0),
                                        stop=(jp == 7), perf_mode=DR)
                                nc.scalar.activation(pre[:, ch, :], ps[:],
                                                     AF.Copy, scale=QS)
                            # RoPE: 4 muls on DVE, 2 fp8 combines on Pool
                            t1 = ropet.tile([128, TPC], F16, tag="t1")
                            t2 = ropet.tile([128, TPC], F16, tag="t2")
                            t3 = ropet.tile([128, TPC], F16, tag="t3")
                            t4 = ropet.tile([128, TPC], F16, tag="t4")
                            nc.vector.tensor_mul(t1[:], pre[:, 0, :],
                                                 rC[:, tcols])
                            nc.vector.tensor_mul(t2[:], pre[:, 1, :],
                                                 rS[:, tcols])
                            nc.vector.tensor_mul(t3[:], pre[:, 0, :],
                                                 rS[:, tcols])
                            nc.vector.tensor_mul(t4[:], pre[:, 1, :],
                                                 rC[:, tcols])
                            nc.gpsimd.tensor_sub(dst[:, 0, tcols], t1[:],
                                                 t2[:])
                            nc.gpsimd.tensor_add(dst[:, 1, tcols], t3[:],
                                                 t4[:])
                        for mt in range(4):
                            ps = psv.tile([128, 256], F32, tag="v")
                            for jp in range(8):
                                nc.tensor.matmul(
                                    ps[:],
                                    hts[jp][:, :, mt * 128:(mt + 1) * 128],
                                    wv_sb[:, 2 * jp:2 * jp + 2, :],
                                    start=(jp == 0), stop=(jp == 7),
                                    perf_mode=DR)
                            nc.scalar.activation(v_sb[:, tb * 4 + mt, :],
                                                 ps[:], AF.Copy, scale=QS)

                    for tb in range(4):
                        qkv_tb(tb)
                    # weave: batch-0 attention of head 0 x qkv for batch 1.
                    at4 = cp.tile([128, SB, TPC], F8, tag="at4", bufs=2)
                    for qb in range(SB):
                        attn_block(0, 0, qb, {2, 5}, poolsA, at4)
                        qkv_tb(4 + qb)
                # qkv pools closed: wider psum rings for remaining attention
                with (
                    tc.tile_pool(name="pssB", bufs=4, space="PSUM") as pssB,
                    tc.tile_pool(name="psoB", bufs=2, space="PSUM") as psoB,
                    tc.tile_pool(name="psdnB", bufs=2, space="PSUM") as psdnB,
                ):
                    poolsB = (pssB, psoB, psdnB)
                    at4 = cp.tile([128, SB, TPC], F8, tag="at4", bufs=2)
                    for qb in (0, 2):
                        attn_two(0, 1, qb, {1, 3, 5}, poolsB, at4)
                    a2a(0)
                    # prefetch: even at_all chunks + first wo pairs during m1
                    for hh in range(2):
                        nc.sync.dma_start(
                            at_all[:, 4 * hh:4 * hh + 4, :],
                            a2a_out_m[0][4 * hh:4 * hh + 4].transpose(
                                [1, 0, 2]))
                    wo_tiles = {}
                    for mp in range(2):
                        ws = wop.tile([128, 2, HC, 128], F8, tag="wo")
                        nc.scalar.dma_start(
                            ws[:].opt(),
                            wo[2 * mp:2 * mp + 2].transpose([1, 0, 2]))
                        wo_tiles[mp] = ws
                    for b in range(B):
                        at4 = cp.tile([128, SB, TPC], F8, tag="at4", bufs=2)
                        for qb in (0, 2):
                            attn_two(1, b, qb, {1, 3, 5}, poolsB, at4)
                    a2a(1)

            # ------------- Stages D+E ----------------------------------------
            with tc.tile_pool(name="late", bufs=1) as late:
              h2x = late.tile([128, HC, 2, TPC], F8, tag="h2x")
              ffx = late.tile([128, FFC, TPC], F8, tag="ffx")
              with (
                tc.tile_pool(name="wop2", bufs=3) as wop2,
                tc.tile_pool(name="lnD", bufs=4) as lnD,
                tc.tile_pool(name="pso2", bufs=3, space="PSUM") as pso2_p,
                tc.tile_pool(name="psstD", bufs=2, space="PSUM") as psstD,
                tc.tile_pool(name="psbcD", bufs=3, space="PSUM") as psbcD,
              ):
                for hh in range(2):
                    nc.sync.dma_start(
                        at_all[:, 8 + 4 * hh:12 + 4 * hh, :],
                        a2a_out_m[1][4 * hh:4 * hh + 4].transpose([1, 0, 2]))
                ps_sx2 = psstD.tile([1, TPC], F32, tag="st")
                ps_sq2 = psstD.tile([1, TPC], F32, tag="st")
                sq2_tiles = {}
                LAG2 = 2

                def stats2(mo):
                    nc.tensor.matmul(ps_sx2[:], ones16[:], x2_sb[:, mo, :],
                                     start=(mo == 0), stop=(mo == HC - 1))
                    nc.tensor.matmul(ps_sq2[:], ones16[:],
                                     sq2_tiles.pop(mo)[:],
                                     start=(mo == 0), stop=(mo == HC - 1))

                for mo in range(HC):
                    mp, s = divmod(mo, 2)
                    if s == 0:
                        if mp in wo_tiles:
                            ws = wo_tiles.pop(mp)
                        else:
                            ws = wop2.tile([128, 2, HC, 128], F8, tag="wo2")
                            nc.scalar.dma_start(
                                ws[:].opt(),
                                wo[2 * mp:2 * mp + 2].transpose([1, 0, 2]))
                        cur_wo = ws
                    ps = pso2_p.tile([128, TPC], F32, tag="o2")
                    for jp in range(8):
                        nc.tensor.matmul(
                            ps[:], cur_wo[:, s, 2 * jp:2 * jp + 2, :],
                            at_all[:, 2 * jp:2 * jp + 2, :],
                            start=(jp == 0), stop=(jp == 7), perf_mode=DR)
                    # x2 = psum * 2^-16 + x  (wo 1024x, at 64x)
                    nc.vector.scalar_tensor_tensor(
                        x2_sb[:, mo, :], ps[:], 2.0 ** -16, x16[:, mo, :],
                        MULT, ADD)
                    sq2 = lnD.tile([128, TPC], F16, tag="sq2")
                    nc.vector.tensor_mul(sq2[:], x2_sb[:, mo, :],
                                         x2_sb[:, mo, :])
                    sq2_tiles[mo] = sq2
                    if mo >= LAG2:
                        stats2(mo - LAG2)
                for mo in range(HC - LAG2, HC):
                    stats2(mo)

                def put_h2(j, ps_c1, ps_c0):
                    t1 = lnD.tile([128, TPC], F16, tag="t1")
                    c0s = lnD.tile([128, TPC], F16, tag="c0s")
                    h2f = lnD.tile([128, TPC], F16, tag="h2f")
                    nc.vector.tensor_mul(t1[:], x2_sb[:, j, :], ps_c1[:])
                    nc.scalar.activation(c0s[:], ps_c0[:], AF.Copy)
                    nc.gpsimd.tensor_add(h2f[:], t1[:], c0s[:])
                    if j % 2 == 0:
                        nc.scalar.activation(h2x[:, j, 0, :], h2f[:], AF.Copy)
                    else:
                        nc.vector.tensor_copy(h2x[:, j, 0, :], h2f[:])
                    nc.gpsimd.tensor_sub(h2x[:, j, 1, :], h2f[:],
                                         h2x[:, j, 0, :])

                _ln_coeff(ps_sx2, ps_sq2, g2b2_sb, put_h2, lnD, psbcD)

              # ------------- Stage E: MLP -------------------------------------
              with (
                tc.tile_pool(name="wf1p", bufs=3) as wf1p,
                tc.tile_pool(name="wf2p", bufs=3) as wf2p,
                tc.tile_pool(name="mlt", bufs=4) as mlt,
                tc.tile_pool(name="psf1", bufs=3, space="PSUM") as psf1_p,
                tc.tile_pool(name="psf2", bufs=3, space="PSUM") as psf2_p,
              ):
                wf1_tiles = {}

                def wf1_load(mp):
                    ws = wf1p.tile([128, 2, HC, 2, 128], F8, tag="wf1")
                    nc.scalar.dma_start(
                        ws[:].opt(),
                        wf1[2 * mp:2 * mp + 2].transpose([1, 0, 2]))
                    wf1_tiles[mp] = ws

                wf1_load(0)
                wf1_load(1)
                for mo in range(FFC):
                    mp, s = divmod(mo, 2)
                    if s == 0:
                        cur_wf1 = wf1_tiles.pop(mp)
                        if 2 * (mp + 2) < FFC:
                            wf1_load(mp + 2)
                    ps = psf1_p.tile([128, TPC], F32, tag="f1")
                    for jp in range(8):  # hi*hi
                        nc.tensor.matmul(
                            ps[:], cur_wf1[:, s, 2 * jp:2 * jp + 2, 1, :],
                            h2x[:, 2 * jp:2 * jp + 2, 0, :],
                            start=(jp == 0), stop=False, perf_mode=DR)
                    for j in range(HC):  # cross: Wl*Ah + Wh*Al
                        nc.tensor.matmul(
                            ps[:], cur_wf1[:, s, j, :, :], h2x[:, j, :, :],
                            start=False, stop=(j == HC - 1), perf_mode=DR)
                    nc.scalar.activation(ffx[:, mo, :], ps[:], AF.Gelu,
                                         scale=1.0 / WS)
                out_pair = [None]
                for mo in range(HC):
                    ws = wf2p.tile([128, FFC, 2, 128], F8, tag="wf2")
                    nc.scalar.dma_start(ws[:].opt(), wf2[mo])
                    ps = psf2_p.tile([128, TPC], F32, tag="f2")
                    for jp in range(FFC):  # 2-term: W hi pairs then W lo pairs
                        nc.tensor.matmul(
                            ps[:], ws[:, jp, :, :],
                            ffx[:, 2 * (jp % 16):2 * (jp % 16) + 2, :],
                            start=(jp == 0), stop=(jp == FFC - 1),
                            perf_mode=DR)
                    if mo % 2 == 0:
                        out_pair[0] = mlt.tile([128, 2, TPC], F32, tag="ot",
                                               bufs=2, name="otp")
                    ot = out_pair[0]
                    nc.vector.scalar_tensor_tensor(
                        ot[:, mo % 2, :], ps[:], 1.0 / WS, x2_sb[:, mo, :],
                        MULT, ADD)
                    if mo == HC - 1:
                        # last pair as two singles: shorter drain tail
                        nc.sync.dma_start(
                            outT[mo - 1, :, :], ot[:, 0, :])
                        nc.sync.dma_start(
                            outT[mo, :, :], ot[:, 1, :])
                    elif mo % 2 == 1:
                        nc.sync.dma_start(
                            outT[mo - 1:mo + 1, :, :].transpose([1, 0, 2]),
                            ot[:])
    return nc


def _build():
    if "nc" in _CACHE:
        return _CACHE["nc"]
    nc = bacc.Bacc(
        "TRN2", target_bir_lowering=False, debug=False,
        enable_asserts=True, num_devices=NCORES,
    )
    _emit(nc)
    nc.compile()
    _CACHE["nc"] = nc
    return nc


def _q8(v):
    return np.asarray(v, np.float32).astype(E4)


def prepare_inputs(x, pe, w_qkv, w_out, w_fc1, w_fc2, g1, b1, g2, b2):
    x = np.asarray(x, np.float32)
    pe = np.asarray(pe, np.float32)
    w_qkv = np.asarray(w_qkv, np.float32)
    w_out = np.asarray(w_out, np.float32)
    w_fc1 = np.asarray(w_fc1, np.float32)
    w_fc2 = np.asarray(w_fc2, np.float32)

    xf = x.reshape(TOK, HID)
    ropeC = np.tile(pe[:, 0::2].T, (2, B)).astype(np.float16)   # [128, TOK]
    ropeS = np.tile(pe[:, 1::2].T, (2, B)).astype(np.float16)
    g1b1 = np.stack([np.asarray(g1, np.float32),
                     np.asarray(b1, np.float32)]).astype(np.float16)
    g2b2 = np.stack([np.asarray(g2, np.float32),
                     np.asarray(b2, np.float32)]).astype(np.float16)

    # out_proj: strips over reordered contraction chunks
    # jj<8 -> head 2*jj (from a2a[0]); jj>=8 -> head 2*(jj-8)+1 (a2a[1])
    head_of = [2 * jj if jj < 8 else 2 * (jj - 8) + 1 for jj in range(HC)]
    wo_h = np.empty((HC, 128, HC * 128), dtype=E4)
    wt = (w_out * WS).astype(np.float32)   # [out, feat]
    for mo in range(HC):
        blk = np.empty((128, HC, 128), np.float32)
        for jj in range(HC):
            h = head_of[jj]
            blk[:, jj, :] = wt[mo * 128:(mo + 1) * 128,
                               h * 128:(h + 1) * 128].T
        wo_h[mo] = _q8(blk.reshape(128, HC * 128))

    def hilo_strips(w, n_strips, kc, stacked=False):
        # stacked=False (3-term): per chunk j the DR pair is (lo[j], hi[j]).
        # stacked=True (2-term): pairs [0, kc/2) are (hi[2k], hi[2k+1]),
        # pairs [kc/2, kc) are (lo[2k], lo[2k+1]); the device pairs both
        # halves with the same hi-only activations.
        w = (w * WS).astype(np.float32)
        hi = _q8(w).astype(np.float32)
        lo = _q8(w - hi).astype(np.float32)
        half = kc // 2
        out = np.empty((n_strips, 128, kc * 2 * 128), dtype=E4)
        for mo in range(n_strips):
            rows = slice(mo * 128, (mo + 1) * 128)
            blk = np.empty((128, kc, 2, 128), np.float32)
            wl = lo[rows]; wh = hi[rows]    # [128(col), kc*128]
            ht = wh.reshape(128, kc, 128).transpose(2, 1, 0)  # [p, j, col]
            lt = wl.reshape(128, kc, 128).transpose(2, 1, 0)
            if stacked:
                for kk in range(half):
                    blk[:, kk, 0, :] = ht[:, 2 * kk]
                    blk[:, kk, 1, :] = ht[:, 2 * kk + 1]
                    blk[:, half + kk, 0, :] = lt[:, 2 * kk]
                    blk[:, half + kk, 1, :] = lt[:, 2 * kk + 1]
            else:
                blk[:, :, 0, :] = lt
                blk[:, :, 1, :] = ht
            out[mo] = _q8(blk.reshape(128, kc * 2 * 128))
        return out

    wf1_h = hilo_strips(w_fc1, FFC, HC)     # [FFC, 128, HC*2*128]
    wf2_h = hilo_strips(w_fc2, HC, FFC, stacked=True)

    in_maps = []
    for c in range(NCORES):
        hsl = slice(2 * c * D, (2 * c + 2) * D)

        def qk_lay(rows):
            # rows [256, HID] (2 heads) -> [128(p), HC(j), 2(ch), 128(m,pp)]
            r = (rows * WS).astype(np.float32)
            t = r.reshape(2, 64, 2, HC, 128)       # [m, pp, ch, j, p]
            t = t.transpose(4, 3, 2, 0, 1)          # [p, j, ch, m, pp]
            return _q8(t.reshape(128, HC * 2 * 128))

        qrows = w_qkv[hsl]
        krows = w_qkv[HID + 2 * c * D: HID + (2 * c + 2) * D]
        vrows = w_qkv[2 * HID + 2 * c * D: 2 * HID + (2 * c + 2) * D]
        vv = (vrows * WS).astype(np.float32).reshape(2, 128, HC, 128)
        wv_c = _q8(vv.transpose(3, 2, 0, 1).reshape(128, HC * 256))

        xTc = np.ascontiguousarray(
            xf[c * TPC:(c + 1) * TPC].T).astype(np.float16)
        in_maps.append({
            "xT": xTc.reshape(HC, 128, TPC),
            "wq": qk_lay(qrows), "wk": qk_lay(krows), "wv": wv_c,
            "wo": wo_h, "wf1": wf1_h, "wf2": wf2_h,
            "g1b1": g1b1, "g2b2": g2b2,
            "ropeC": ropeC, "ropeS": ropeS,
        })
    return in_maps


def run(in_maps, **kwargs):
    nc = _build()
    return bass_utils.run_bass_kernel_spmd(
        nc, in_maps, core_ids=list(range(NCORES)), **kwargs
    )


def kernel(x, pe, w_qkv, w_out, w_fc1, w_fc2, g1, b1, g2, b2):
    in_maps = prepare_inputs(x, pe, w_qkv, w_out, w_fc1, w_fc2, g1, b1, g2, b2)
    res = run(in_maps)
    fullT = np.concatenate(
        [res.results[c]["outT"].reshape(HID, TPC) for c in range(NCORES)],
        axis=1)
    return np.ascontiguousarray(fullT.T).reshape(B, S, HID).astype(np.float32)

